# revision 7
# baseline (speedup 1.0000x reference)
"""Trainium2 Bass kernel for the MACE-style equivariant Contraction module.

Math: the reference einsum chain collapses, per (b, c) pair, to three
polynomial terms in the 16-vector x[b, c, :]:

  out[b,c,w] = sum_k W3[b,k,c] * sum_{m3} A3[m3, w, k] * mono3[b,c, m3]   (deg-3)
             + sum_k W2[b,k,c] * sum_{m2} A2[m2, w, k] * mono2[b,c, m2]   (deg-2)
             +       W1[b,0,c] * sum_{a}  U1[w, a, 0]  * x[b,c,a]         (deg-1)

where mono3 / mono2 are the 816 / 136 unique monomials x_a x_b x_c (a<=b<=c)
/ x_a x_b (a<=b), A3/A2 are the U tensors symmetrized into that monomial
basis (exact — only the symmetric part of U contributes), and
W{nu}[b,k,c] = sum_e y[b,e] w{nu}[e,k,c].

Device strategy (8 cores, data-parallel over nodes b):
  - FEAT[bc, 1024] = [mono3 816 | mono2 136 | x 16 | zero pad 56] in bf16,
    built on the vector engine with broadcast-AP tensor_tensor ops,
    16 nodes per chunk.
  - FEAT is staged through DRAM and read back with the DMA xbar transpose
    to get K-major tiles FT[m 128, bc 512].
  - One PSUM-accumulated matmul chain per 128 bc rows:
    R[bc, 196] = sum_j FT_j^T @ BIG_j  (BIG = [A3|A2|A1] packed, 8 K-chunks).
  - Per-node combine on DVE: multiply R by the W weights (per-partition,
    broadcast APs) and reduce over k.
"""

import os
from contextlib import ExitStack
from itertools import permutations

import numpy as np
import ml_dtypes

import concourse.bass as bass
import concourse.bacc as bacc
import concourse.tile as tile
from concourse import mybir
from concourse.bass_utils import run_bass_kernel_spmd

BF16 = ml_dtypes.bfloat16

N_CORES = 8
B, C, E, L, Q = 1024, 128, 10, 16, 7
P3, P2, P1 = 23, 4, 1
BPC = B // N_CORES          # 128 nodes per core
NB = 16                     # nodes per FEAT build chunk
NCHUNK = BPC // NB          # 8 chunks per core
M3OFF, M2OFF, XOFF, PADOFF = 0, 816, 952, 968
FEATW = 1024                # padded feature width (8 K-chunks of 128)
KCH = FEATW // 128
N3, N2, N1 = Q * P3, Q * P2, Q * P1          # 161, 28, 7
NOUT = N3 + N2 + N1                          # 196
NK = P3 + P2 + P1                            # 28 weight columns
BCT = 4                     # nodes per phase-2 tile (512 bc rows)

_TRI = [(c + 1) * (c + 2) // 2 for c in range(L)]            # T_c
_OFF3 = np.concatenate([[0], np.cumsum(_TRI)])               # offsets of c3 blocks

# Results of the last device run (for test harnesses).
LAST_EXEC_NS = None
LAST_TRACE = None

_PROGRAM_CACHE = {}


# ----------------------------------------------------------------- host prep

def _build_bigmat(U3, U2, U1):
    """[1024, 196] f32: rows = FEAT entries, cols = [w*23+k | w*4+k | w]."""
    U3 = np.asarray(U3, np.float64)
    U2 = np.asarray(U2, np.float64)
    U1 = np.asarray(U1, np.float64)
    BIG = np.zeros((FEATW, NOUT), np.float64)
    # degree 3: monomial (a <= bp <= c3), row = off3[c3] + bp(bp+1)/2 + a
    rows, cols_a, cols_b, cols_c, mult = [], [], [], [], []
    for c3 in range(L):
        for bp in range(c3 + 1):
            for a in range(bp + 1):
                m = _OFF3[c3] + bp * (bp + 1) // 2 + a
                for (p, q, r) in set(permutations((a, bp, c3))):
                    rows.append(m)
                    cols_a.append(p)
                    cols_b.append(q)
                    cols_c.append(r)
    np.add.at(
        BIG[:, :N3].reshape(FEATW, Q, P3),
        (np.array(rows),),
        U3[:, np.array(cols_a), np.array(cols_b), np.array(cols_c), :].transpose(
            1, 0, 2
        ),
    )
    # degree 2
    rows2, pa, pb = [], [], []
    for bp in range(L):
        for a in range(bp + 1):
            m2 = bp * (bp + 1) // 2 + a
            for (p, q) in set(permutations((a, bp))):
                rows2.append(M2OFF + m2)
                pa.append(p)
                pb.append(q)
    np.add.at(
        BIG[:, N3:N3 + N2].reshape(FEATW, Q, P2),
        (np.array(rows2),),
        U2[:, np.array(pa), np.array(pb), :].transpose(1, 0, 2),
    )
    # degree 1
    BIG[XOFF:XOFF + L, N3 + N2:] = U1[:, :, 0].T
    return BIG.astype(np.float32)


def _host_prep(x, y, U3, U2, U1, w3, w2, w1):
    """Returns (big_packed, xt_cores, wt_cores) as bf16 numpy arrays."""
    BIG = _build_bigmat(U3, U2, U1)
    # pack [128, KCH*196]: row p, chunk j  <-  BIG row j*128+p
    big_packed = np.ascontiguousarray(
        BIG.reshape(KCH, 128, NOUT).transpose(1, 0, 2).reshape(128, KCH * NOUT)
    ).astype(BF16)

    # per-node weights W[b, k, c]; device layout [c, k, b_local]
    W = np.concatenate(
        [
            np.einsum("be,ekc->bkc", y, w3, optimize=True),
            np.einsum("be,ekc->bkc", y, w2, optimize=True),
            np.einsum("be,ekc->bkc", y, w1, optimize=True),
        ],
        axis=1,
    ).astype(np.float32)                                   # [B, 28, C]
    xt = np.ascontiguousarray(np.asarray(x, np.float32).transpose(1, 0, 2))  # [C,B,L]

    xt_cores, wt_cores = [], []
    for r in range(N_CORES):
        sl = slice(r * BPC, (r + 1) * BPC)
        xt_cores.append(
            np.ascontiguousarray(xt[:, sl, :]).reshape(C, BPC * L).astype(BF16)
        )
        wt_cores.append(
            np.ascontiguousarray(W[sl].transpose(2, 1, 0)).reshape(
                C, NK * BPC
            ).astype(BF16)
        )
    return big_packed, xt_cores, wt_cores


# ------------------------------------------------------------- device program

def _emit_program():
    nc = bacc.Bacc("TRN2", target_bir_lowering=False, debug=False,
                   num_devices=N_CORES)
    xt_d = nc.dram_tensor("xt", [C, BPC * L], mybir.dt.bfloat16,
                          kind="ExternalInput")
    wt_d = nc.dram_tensor("wt", [C, NK * BPC], mybir.dt.bfloat16,
                          kind="ExternalInput")
    big_d = nc.dram_tensor("big", [128, KCH * NOUT], mybir.dt.bfloat16,
                           kind="ExternalInput")
    out_d = nc.dram_tensor("out", [BPC, C * Q], mybir.dt.float32,
                           kind="ExternalOutput")

    with tile.TileContext(nc) as tc:
        with ExitStack() as ctx:
            _emit_kernel(ctx, tc, xt_d, wt_d, big_d, out_d)
    nc.compile()
    return nc


def _emit_kernel(ctx, tc, xt_d, wt_d, big_d, out_d):
    nc = tc.nc
    f32 = mybir.dt.float32
    bf16 = mybir.dt.bfloat16
    mult = mybir.AluOpType.mult
    add = mybir.AluOpType.add

    singles = ctx.enter_context(tc.tile_pool(name="singles", bufs=1))
    featp = ctx.enter_context(tc.tile_pool(name="featp", bufs=2))
    dramp = ctx.enter_context(tc.tile_pool(name="dramp", bufs=1, space="DRAM"))
    ftp = ctx.enter_context(tc.tile_pool(name="ftp", bufs=6))
    psump = ctx.enter_context(tc.tile_pool(name="psump", bufs=8, space="PSUM"))
    cmbp = ctx.enter_context(tc.tile_pool(name="cmbp", bufs=4))
    outp = ctx.enter_context(tc.tile_pool(name="outp", bufs=2))

    xt = singles.tile([C, BPC * L], bf16)
    wt = singles.tile([C, NK * BPC], bf16)
    big = singles.tile([128, KCH * NOUT], bf16)
    nc.sync.dma_start(out=xt[:], in_=xt_d.ap())
    nc.sync.dma_start(out=wt[:], in_=wt_d.ap())
    nc.sync.dma_start(out=big[:], in_=big_d.ap())

    featdram = dramp.tile([BPC * C, FEATW], bf16)

    xv = xt[:].rearrange("p (b i) -> p b i", i=L)       # [C, BPC, L]

    for ch in range(NCHUNK):
        b0 = ch * NB
        feat = featp.tile([C, NB * FEATW], bf16)
        fv = feat[:].rearrange("p (b m) -> p b m", m=FEATW)   # [C, NB, FEATW]
        xc = xv[:, b0:b0 + NB, :]                             # [C, NB, L]

        # mono2 block: cols M2OFF + bp(bp+1)/2 + a for a<=bp
        for bp in range(L):
            w = bp + 1
            c0 = M2OFF + bp * (bp + 1) // 2
            nc.vector.tensor_tensor(
                fv[:, :, c0:c0 + w],
                xc[:, :, 0:w],
                xc[:, :, bp:bp + 1].broadcast_to([C, NB, w]),
                mult,
            )
        # mono3 blocks: cols off3[c3] + (pair idx), = mono2[:Tc3] * x_c3
        for c3 in range(L):
            w = _TRI[c3]
            nc.vector.tensor_tensor(
                fv[:, :, _OFF3[c3]:_OFF3[c3] + w],
                fv[:, :, M2OFF:M2OFF + w],
                xc[:, :, c3:c3 + 1].broadcast_to([C, NB, w]),
                mult,
            )
        # x block + zero pad
        nc.vector.tensor_copy(fv[:, :, XOFF:XOFF + L], xc)
        nc.vector.memset(fv[:, :, PADOFF:FEATW], 0.0)

        # stage chunk to DRAM: row (b*C + c), col m
        dst = bass.AP(
            tensor=featdram.tensor,
            offset=featdram[:].offset + b0 * C * FEATW,
            ap=[[FEATW, C], [C * FEATW, NB], [1, FEATW]],
        )
        nc.sync.dma_start(out=dst, in_=fv)

        # ---------------- phase 2: 4 tiles of BCT nodes each ----------------
        for t in range(NB // BCT):
            r0 = (b0 + t * BCT) * C
            psums = [
                psump.tile([128, NOUT], f32, name=f"psum{s}", tag=f"psum{s}",
                           bufs=2)
                for s in range(BCT)
            ]
            for j in range(KCH):
                ft = ftp.tile([128, BCT * C], bf16)
                src = bass.AP(
                    tensor=featdram.tensor,
                    offset=featdram[:].offset + r0 * FEATW + j * 128,
                    ap=[[FEATW, BCT * C], [1, 128]],
                )
                nc.sync.dma_start_transpose(ft[:], src)
                for s in range(BCT):
                    nc.tensor.matmul(
                        psums[s][:],
                        lhsT=ft[:, s * C:(s + 1) * C],
                        rhs=big[:, j * NOUT:(j + 1) * NOUT],
                        start=(j == 0),
                        stop=(j == KCH - 1),
                    )
            if t == 0:
                outacc = outp.tile([C, NB * Q], f32)
            for s in range(BCT):
                b_loc = b0 + t * BCT + s
                ps = psums[s]
                part = wt[:].ap[0]
                w3ap = bass.AP(tensor=wt.tensor,
                               offset=wt[:].offset + b_loc,
                               ap=[part, [0, Q], [BPC, P3]])
                w2ap = bass.AP(tensor=wt.tensor,
                               offset=wt[:].offset + P3 * BPC + b_loc,
                               ap=[part, [0, Q], [BPC, P2]])
                w1ap = bass.AP(tensor=wt.tensor,
                               offset=wt[:].offset + (P3 + P2) * BPC + b_loc,
                               ap=[part, [0, Q]])
                t3 = cmbp.tile([C, N3], f32, tag="t3")
                nc.vector.tensor_tensor(
                    t3[:].rearrange("p (w k) -> p w k", k=P3),
                    ps[:, 0:N3].rearrange("p (w k) -> p w k", k=P3),
                    w3ap, mult)
                r3 = cmbp.tile([C, Q], f32, tag="r3")
                nc.vector.reduce_sum(
                    out=r3[:],
                    in_=t3[:].rearrange("p (w k) -> p w k", k=P3),
                    axis=mybir.AxisListType.X)
                t2 = cmbp.tile([C, N2], f32, tag="t2")
                nc.vector.tensor_tensor(
                    t2[:].rearrange("p (w k) -> p w k", k=P2),
                    ps[:, N3:N3 + N2].rearrange("p (w k) -> p w k", k=P2),
                    w2ap, mult)
                r2 = cmbp.tile([C, Q], f32, tag="r2")
                nc.vector.reduce_sum(
                    out=r2[:],
                    in_=t2[:].rearrange("p (w k) -> p w k", k=P2),
                    axis=mybir.AxisListType.X)
                r1 = cmbp.tile([C, Q], f32, tag="r1")
                nc.vector.tensor_tensor(
                    r1[:], ps[:, N3 + N2:NOUT], w1ap, mult)
                r32 = cmbp.tile([C, Q], f32, tag="r32")
                nc.vector.tensor_tensor(r32[:], r3[:], r2[:], add)
                o_sl = outacc[:, (t * BCT + s) * Q:(t * BCT + s + 1) * Q]
                nc.vector.tensor_tensor(o_sl, r32[:], r1[:], add)

        # write chunk output: out row b_loc, col c*7+w
        odst = bass.AP(
            tensor=out_d,
            offset=b0 * C * Q,
            ap=[[Q, C], [C * Q, NB], [1, Q]],
        )
        nc.sync.dma_start(
            out=odst,
            in_=outacc[:].rearrange("p (b w) -> p b w", w=Q),
        )


def _get_program():
    if "nc" not in _PROGRAM_CACHE:
        _PROGRAM_CACHE["nc"] = _emit_program()
    return _PROGRAM_CACHE["nc"]


def _install_ntff_shim():
    """bass_utils expects antenv.axon_hooks for NTFF capture under axon;
    this container's antenv lacks it — recreate from trn_agent_boot."""
    try:
        from antenv.axon_hooks import get_axon_ntff_profile_hook  # noqa: F401
        return
    except ImportError:
        pass
    try:
        import sys
        import types

        from trn_agent_boot.trn_boot import _ntff_profile_via_ctypes

        hook = _ntff_profile_via_ctypes("/opt/axon/libaxon_pjrt.so")
        mod = types.ModuleType("antenv.axon_hooks")
        mod.get_axon_ntff_profile_hook = lambda: hook
        sys.modules["antenv.axon_hooks"] = mod
        import antenv

        antenv.axon_hooks = mod
    except Exception:
        pass


# ------------------------------------------------------------------ entry

def kernel(x, y, U3, U2, U1, w3, w2, w1):
    global LAST_EXEC_NS, LAST_TRACE
    big_packed, xt_cores, wt_cores = _host_prep(x, y, U3, U2, U1, w3, w2, w1)
    nc = _get_program()
    in_maps = [
        {"xt": xt_cores[r], "wt": wt_cores[r], "big": big_packed}
        for r in range(N_CORES)
    ]
    trace = bool(int(os.environ.get("KERNEL_TRACE", "0")))
    if trace:
        _install_ntff_shim()
        try:
            res = run_bass_kernel_spmd(
                nc, in_maps, core_ids=list(range(N_CORES)), trace=True,
            )
        except Exception as e:  # trace capture is best-effort
            print(f"trace capture failed ({type(e).__name__}: {e}); "
                  "re-running untraced")
            res = run_bass_kernel_spmd(
                nc, in_maps, core_ids=list(range(N_CORES)), trace=False,
            )
    else:
        res = run_bass_kernel_spmd(
            nc, in_maps, core_ids=list(range(N_CORES)), trace=False,
        )
    LAST_EXEC_NS = res.exec_time_ns
    LAST_TRACE = res.instructions_and_trace
    out = np.concatenate([res.results[r]["out"] for r in range(N_CORES)], axis=0)
    return out.astype(np.float32)
